# revision 15
# baseline (speedup 1.0000x reference)
"""Trainium2 Bass kernel for the ExportableStudentSNN1d problem.

v2: z-domain LIF (no per-step bias add), sigmoid-as-step spike masks with
free accum counting, {0,1} s1 encoding from the Pool engine.

Data-parallel over batch: 64 samples -> 8 cores x 8 samples. Each core runs
an identical NEFF on its batch shard; host concatenates the [8, 4] outputs.

Math notes (TAU1 = 1.0 makes layer-1 LIF memoryless):
  s1_t = (conv1(x_t)*G + b1*G >= TH1)  <=>  conv1(x_t) >= TH1/G - b1
  s1 stored {0,1} fp8 via Pool-engine is_ge (exact).
  Layer 2 runs in a 2^4-scaled, bias-shifted domain. With y = SC2*v2 and
  beta = A*b2 (A = (10/9)*G*SC2), define z = y - beta. Then
     z_t = conv2(s1_t; W2*A fp8) + alpha*chat_{t-1}        [all on PE]
     m_t = [z_t < theta - beta]   (theta = TH2*SC2)
         = Sigmoid(-BIG*(z_t - (theta-beta)))   on ACT; exact {0,1} by
           saturation for |arg|>=160 (measured); accum_out counts sum(m)
           per row -> spike counts for free.
     chat_t = (z_t + beta)*m_t    via scalar_tensor_tensor, fp8.
  alpha = fp8(-1/9) = -7/64 rides slot 1 of the tap-8 DoubleRow pair as
  alpha*I with chat as that slot's moving operand, so the LIF decay +
  carry recursion cost zero vector-engine work beyond the chat op.
  fp8 weight-quantization DC error is cancelled by folding
  -sum_{ci,k} dW[co,ci,k] * E[s1[ci]] into b2 (E[s1[ci]] analytic: conv1
  output is Gaussian with sigma = ||W1[ci]||_F).
  spikes = 1 - m, so out = (bfc + rowsum(Wfc)) - (Wfc @ sum(m))/(T*L).

conv2 is 5 fp8 DoubleRow matmuls per 512-chunk: taps (0,1),(2,3),(4,5),
(6,7) on slabs {0,2} + (tap8, alpha*I) on slabs {0,1+2h}. s1 tile is
[128, 4, S1P]: [s1, chat(h0), s1<<1, chat(h1)]; slab2 = slab0 shifted one
column via SBUF->SBUF DMA on the (otherwise idle) qSP queue; x staging
rides qAct so the per-step shift never queues behind a 4.5MB transfer.
conv1 is pipelined TWO steps ahead so the shift DMA latency (~2-4us) is
hidden at the ~2.6us steady-state cycle time. LIF ops are split per
512-chunk so the chat -> conv2(t+1) recursion never blocks the PE; the
h1/c1 chat chunk runs on Pool to keep DVE under the PE's cycle budget.
conv1 stays bf16 with a DMA-materialized im2col.
"""

import math

import numpy as np
import ml_dtypes

import concourse.bacc as bacc
import concourse.tile as tile
import concourse.mybir as mybir
from concourse.bass_utils import run_bass_kernel_spmd

F32 = mybir.dt.float32
BF16 = mybir.dt.bfloat16
FP8 = mybir.dt.float8e4
E4 = ml_dtypes.float8_e4m3

N_CORES = 8
B, C_IN, L, T = 64, 12, 2048, 20
C1, C2, K, PAD = 128, 256, 9, 4
GAIN, TAU2, TH1, TH2 = 3.0, 0.9, 0.02, 0.02
NCLS = 4
B_SH = B // N_CORES            # 8 samples per core
LH = 1024                      # L processed in halves
HALO = 8                       # x halo per side (conv1 then conv2 shifts)
S1W = LH + 2 * PAD             # 1032 s1 columns needed per L-half
S1P = 1040                     # s1 slab width (mult of 16)
A2S = (10.0 / 9.0) * GAIN      # 10/3: multiplier on conv2 psum
SC2 = 16.0                     # layer-2 scale; keeps z, chat in fp8 range
ALPHA = -7.0 / 64.0            # fp8(-1/9): carry decay inside the PE
BIG = 65536.0                  # sigmoid step sharpness (layer 2)
BIG1 = float(2 ** 26)          # sigmoid step sharpness (layer 1; narrower
                               # fractional band since s1 feeds 1152 taps)

_CACHE = {}


def _build():
    nc = bacc.Bacc("TRN2", target_bir_lowering=False, debug=False)

    # x arrives HOST-side im2col'd: row (12k+ci) of [b, lh] holds
    # x[b, ci, t, lh*1024 + c + k - 8] (zero-padded at L edges), so staging
    # a segment is ONE contiguous 4.5MB DMA instead of thousands of 2KB
    # descriptors that saturate the rings.
    x_d = nc.dram_tensor(
        "x", [B_SH, 2, K * C_IN, T * S1W], BF16, kind="ExternalInput")
    w1t_d = nc.dram_tensor("w1t", [K * C_IN, C1], BF16, kind="ExternalInput")
    w2dr_d = nc.dram_tensor("w2dr", [C1, 2, 8 * C1], FP8, kind="ExternalInput")
    w285_d = nc.dram_tensor("w285", [C1, 2, C2], FP8, kind="ExternalInput")
    nth1s_d = nc.dram_tensor("nth1s", [C1, 1], F32, kind="ExternalInput")
    beta_d = nc.dram_tensor("beta", [C1, 2], F32, kind="ExternalInput")
    sigb_d = nc.dram_tensor("sigb", [C1, 2], F32, kind="ExternalInput")
    wfc_d = nc.dram_tensor("wfc", [C1, 2 * NCLS], F32, kind="ExternalInput")
    bfc_d = nc.dram_tensor("bfc", [NCLS, 1], F32, kind="ExternalInput")
    out_d = nc.dram_tensor("out", [B_SH, NCLS], F32, kind="ExternalOutput")

    with tile.TileContext(nc) as tc:
        with (
            tc.tile_pool(name="const", bufs=1) as cpool,
            tc.tile_pool(name="xstage", bufs=2) as xpool,
            tc.tile_pool(name="s1", bufs=4) as s1pool,
            tc.tile_pool(name="lif", bufs=4) as lifpool,
            tc.tile_pool(name="psum1", bufs=1, space="PSUM") as pp1,
            tc.tile_pool(name="psum2", bufs=2, space="PSUM") as pp2,
            tc.tile_pool(name="psfc", bufs=1, space="PSUM") as ppfc,
        ):
            # ---- constants / weights (resident) ----
            # w1t rows (12k+ci) hold W1[:, ci, k] (im2col layout)
            w1t = cpool.tile([K * C_IN, C1], BF16)
            nc.sync.dma_start(w1t[:], w1t_d.ap())
            # DR pairs: w2dr[ci, i, (j*2+h)*128+co] = W2q[h*128+co, ci, 2j+i]
            w2dr = cpool.tile([C1, 2, 8 * C1], FP8)
            nc.sync.dma_start(w2dr[:], w2dr_d.ap())
            # 5th pair: slot0 = tap 8 of W2q, slot1 = alpha*I (carry decay)
            w285 = cpool.tile([C1, 2, C2], FP8)
            nc.sync.dma_start(w285[:], w285_d.ap())
            nth1s = cpool.tile([C1, 1], F32)
            nc.sync.dma_start(nth1s[:], nth1s_d.ap())
            beta = cpool.tile([C1, 2], F32)
            nc.sync.dma_start(beta[:], beta_d.ap())
            sigb = cpool.tile([C1, 2], F32)
            nc.sync.dma_start(sigb[:], sigb_d.ap())
            wfc = cpool.tile([C1, 2 * NCLS], F32)
            nc.sync.dma_start(wfc[:], wfc_d.ap())
            bfc = cpool.tile([NCLS, 1], F32)
            nc.sync.dma_start(bfc[:], bfc_d.ap())
            # m01 sums, one column per (h, b, lh, t)
            NACC = 2 * B_SH * 2 * T
            acc = cpool.tile([C1, NACC], F32)

            segs = [(b, lh) for b in range(B_SH) for lh in range(2)]
            n_items = len(segs) * T

            def stage_segment(idx):
                # host-side im2col: one contiguous DMA per segment, on the
                # qAct queue so the per-step s1 shift (qSP) never queues
                # behind it
                b, lh = segs[idx]
                xs = xpool.tile([K * C_IN, T * S1W], BF16)
                src = x_d.ap()[b, lh]
                # split every segment: the first conv1 needs only t=0/1,
                # so it must not wait on the full 4.5MB transfer's semaphore
                nc.scalar.dma_start(xs[:, 0 : 2 * S1W], src[:, 0 : 2 * S1W])
                nc.scalar.dma_start(xs[:, 2 * S1W :], src[:, 2 * S1W :])
                return xs

            def conv1_mm(xs, t):
                # conv1: K=108 bf16 matmuls per chunk, emitted EARLY in the
                # cycle so the PE does them before conv2
                p1 = pp1.tile([C1, 1536], F32)
                for c0, cn in ((0, 512), (512, 512), (1024, S1W - 1024)):
                    nc.tensor.matmul(
                        p1[:, c0 : c0 + cn],
                        w1t[:],
                        xs[:, t * S1W + c0 : t * S1W + c0 + cn],
                        start=True,
                        stop=True,
                    )
                return p1

            def s1_make(p1, init_zero=False):
                # s1 slab0 = {0,1} fp8 sigmoid step (exact by saturation).
                # Emitted AFTER the cycle's m01 ops so it sits last in the
                # ACT queue (it has 2 cycles of slack; the m01s have none).
                # slab2 = slab0 shifted one column via qSP SBUF->SBUF DMA
                # (2 cycles of runway from the 2-ahead pipeline).
                # Cols >= 1032 are junk but never read by conv2.
                s1 = s1pool.tile([C1, 4, S1P], FP8)
                if init_zero:
                    # only the first prologue tile needs memsets; later
                    # segment-start tiles get their carry slabs zeroed by
                    # the t=19 z*0 writes
                    nc.gpsimd.memset(s1[:, 1], 0.0)
                    nc.gpsimd.memset(s1[:, 3], 0.0)
                nc.scalar.activation(
                    s1[:, 0], p1[:, 0:S1P],
                    mybir.ActivationFunctionType.Sigmoid,
                    bias=nth1s[:], scale=BIG1,
                )
                nc.sync.dma_start(s1[:, 2, 0 : S1P - 1], s1[:, 0, 1:S1P])
                return s1

            def conv2_block(s1, h):
                # 5 DR matmuls per 512-chunk: 4 tap pairs on slabs {0,2} +
                # the (tap8, alpha*I) pair on slabs {0, 1+2h} -- slot 1's
                # moving operand is the previous step's carry chat(h).
                p2 = pp2.tile([C1, LH], F32)
                for c0 in (0, 512):
                    for j in range(4):
                        nc.tensor.matmul(
                            p2[:, c0 : c0 + 512],
                            w2dr[:, 0:2, (j * 2 + h) * C1 : (j * 2 + h + 1) * C1],
                            s1[:, 0:3:2, c0 + 2 * j : c0 + 2 * j + 512],
                            start=(j == 0),
                            stop=False,
                            perf_mode=mybir.MatmulPerfMode.DoubleRow,
                        )
                    nc.tensor.matmul(
                        p2[:, c0 : c0 + 512],
                        w285[:, 0:2, h * C1 : (h + 1) * C1],
                        (s1[:, 0:2:1, c0 + 8 : c0 + 8 + 512] if h == 0
                         else s1[:, 0:4:3, c0 + 8 : c0 + 8 + 512]),
                        start=False,
                        stop=True,
                        perf_mode=mybir.MatmulPerfMode.DoubleRow,
                    )
                return p2

            def lif_step(p2, s1_next, h, col, zero=False):
                # m = sigmoid-step on ACT, full width (accum -> counts);
                # chat = (z + beta)*m on DVE per 512-chunk (so the
                # chat -> conv2(t+1) recursion never blocks the PE),
                # written into the NEXT s1 tile's carry slab at +8
                # (tap-8 window alignment).
                m01 = lifpool.tile([C1, LH], F32)
                nc.scalar.activation(
                    m01[:], p2[:],
                    mybir.ActivationFunctionType.Sigmoid,
                    bias=sigb[:, h : h + 1],
                    scale=-BIG,
                    accum_out=acc[:, col + h * (NACC // 2) :
                                  col + h * (NACC // 2) + 1],
                )
                if s1_next is None:
                    return
                for c0 in (0, 512):
                    dst = s1_next[:, 1 + 2 * h, 8 + c0 : 8 + c0 + 512]
                    if zero:
                        # segment start: v2 resets, write exact zeros
                        nc.vector.tensor_scalar(
                            dst, p2[:, c0 : c0 + 512], 0.0, None,
                            op0=mybir.AluOpType.mult,
                        )
                    else:
                        nc.vector.scalar_tensor_tensor(
                            dst, p2[:, c0 : c0 + 512],
                            beta[:, h : h + 1], m01[:, c0 : c0 + 512],
                            op0=mybir.AluOpType.add,
                            op1=mybir.AluOpType.mult,
                        )

            # conv1 runs TWO items ahead of conv2/LIF so the s1 sigmoid and
            # the qSP shift DMA are always done before the PE needs slab0/2
            staged = {0: stage_segment(0)}
            tiles = []

            def emit_conv1_mm(j):
                jdx, jt = divmod(j, T)
                return conv1_mm(staged[jdx], jt)

            tiles.append(s1_make(emit_conv1_mm(0), init_zero=True))
            tiles.append(s1_make(emit_conv1_mm(1)))
            for idx in range(len(segs)):
                b, lh = segs[idx]
                if idx + 1 < len(segs):
                    staged[idx + 1] = stage_segment(idx + 1)
                staged.pop(idx - 1, None)
                for t in range(T):
                    i = idx * T + t
                    p1_next = emit_conv1_mm(i + 2) if i + 2 < n_items else None
                    s1_cur = tiles[i]
                    s1_next = tiles[i + 1] if i + 1 < n_items else None
                    tiles[i] = None
                    col = b * (2 * T) + lh * T + t
                    last_t = t == T - 1
                    p2_0 = conv2_block(s1_cur, 0)
                    lif_step(p2_0, s1_next, 0, col, zero=last_t)
                    p2_1 = conv2_block(s1_cur, 1)
                    lif_step(p2_1, s1_next, 1, col, zero=last_t)
                    if p1_next is not None:
                        tiles.append(s1_make(p1_next))

            # ---- pooling + FC head ----
            # pooled = sum over (lh, t, chunk) of m01 row-sums per (h, b)
            pooled = cpool.tile([C1, 2 * B_SH], F32)
            nc.vector.tensor_reduce(
                pooled[:],
                acc[:].rearrange("p (h b c) -> p (h b) c", h=2, b=B_SH),
                axis=mybir.AxisListType.X, op=mybir.AluOpType.add,
            )
            pfc = ppfc.tile([NCLS, B_SH], F32)
            for h in range(2):
                nc.tensor.matmul(
                    pfc[:],
                    wfc[:, h * NCLS : (h + 1) * NCLS],
                    pooled[:, h * B_SH : (h + 1) * B_SH],
                    start=(h == 0),
                    stop=(h == 1),
                )
            # pfc holds Wfc @ sum(m); spikes = 1 - m is folded into scale
            # and the host-adjusted bias: out = bfcc - pfc/(T*L)
            fin = cpool.tile([NCLS, B_SH], F32)
            nc.scalar.activation(
                fin[:], pfc[:], mybir.ActivationFunctionType.Identity,
                bias=bfc[:], scale=-1.0 / float(T * L),
            )
            nc.sync.dma_start(out_d.ap().rearrange("b c -> c b"), fin[:])

    nc.compile()
    return nc


def _prep_consts(W1, b1, W2, b2, Wfc, bfc):
    # w1t im2col layout: row (12k+ci), col co = W1[co, ci, k]
    w1t = np.ascontiguousarray(W1.transpose(2, 1, 0)).reshape(K * C_IN, C1)
    # W2 pre-scaled to the 2^4 domain, fp8-e4m3 quantized
    A = A2S * SC2
    w2q8 = (W2.astype(np.float64) * A).astype(np.float32).astype(E4)
    wt = np.ascontiguousarray(w2q8.transpose(1, 2, 0))  # [C1, K, C2]
    w2dr = np.zeros((C1, 2, 8 * C1), dtype=E4)
    for j in range(4):
        for i in range(2):
            for h in range(2):
                w2dr[:, i, (j * 2 + h) * C1 : (j * 2 + h + 1) * C1] = (
                    wt[:, 2 * j + i, h * C1 : (h + 1) * C1])
    # 5th pair: slot0 = tap 8 (on s1), slot1 = alpha*I (on carry chat)
    w285 = np.zeros((C1, 2, C2), dtype=E4)
    w285[:, 0, :] = wt[:, 8, :]
    eye = (np.eye(C1, dtype=np.float32) * ALPHA).astype(E4)
    w285[:, 1, 0:C1] = eye
    w285[:, 1, C1:C2] = eye
    # DC correction: E[s1[ci]] = Phi((b1 - TH1/G)/sigma), sigma = ||W1[ci]||
    sig = np.sqrt((W1.astype(np.float64) ** 2).sum(axis=(1, 2)))
    z = (b1.astype(np.float64) - TH1 / GAIN) / sig
    p_ci = np.array([0.5 * (1.0 + math.erf(v / math.sqrt(2.0))) for v in z])
    dw = w2q8.astype(np.float64) / A - W2.astype(np.float64)
    b2c = b2.astype(np.float64) - np.einsum("oik,i->o", dw, p_ci)
    # layer-1 threshold: s1 = [conv1 >= TH1/G - b1] via sigmoid step
    nth1s = (-BIG1 * (TH1 / GAIN - b1)).reshape(C1, 1).astype(np.float32)
    beta_full = (A * b2c).astype(np.float32)              # [C2]
    theta = TH2 * SC2
    sigb_full = (BIG * (theta - beta_full)).astype(np.float32)
    beta = beta_full.reshape(2, C1).T.copy()              # [128, 2]
    sigb = sigb_full.reshape(2, C1).T.copy()              # [128, 2]
    wfcT = Wfc.T.reshape(2, C1, NCLS)                     # [2, 128, 4]
    wfc_t = wfcT.transpose(1, 0, 2).reshape(C1, 2 * NCLS).copy()
    # spikes = 1 - m folded into the FC epilogue:
    # out = (bfc + rowsum(Wfc)) - (Wfc @ sum_m)/(T*L)
    bfc_c = (bfc + Wfc.sum(axis=1)).reshape(NCLS, 1).astype(np.float32)
    return {
        "w1t": w1t.astype(ml_dtypes.bfloat16),
        "w2dr": w2dr,
        "w285": w285,
        "nth1s": nth1s,
        "beta": beta,
        "sigb": sigb,
        "wfc": wfc_t.astype(np.float32),
        "bfc": bfc_c,
    }


def kernel(x, W1, b1, W2, b2, Wfc, bfc, _trace=False):
    x = np.asarray(x, dtype=np.float32)
    # [B, Cin, L, T] -> [B, Cin, T, L] bf16 so on-chip reads are unit-stride
    x_t = np.ascontiguousarray(x.transpose(0, 1, 3, 2)).astype(ml_dtypes.bfloat16)
    # host-side im2col: xim[b, lh, 12k+ci, t, c] = x[b, ci, t, lh*LH+c+k-8]
    # (zero outside [0, L)) -- turns on-chip staging into one big DMA
    xim = np.zeros((B, 2, K * C_IN, T, S1W), dtype=ml_dtypes.bfloat16)
    for lh in range(2):
        l0 = lh * LH
        for k in range(K):
            c_lo = max(0, HALO - k - l0)
            c_hi = min(S1W, L - l0 - k + HALO)
            xim[:, lh, C_IN * k : C_IN * (k + 1), :, c_lo:c_hi] = (
                x_t[:, :, :, l0 + c_lo + k - HALO : l0 + c_hi + k - HALO])
    xim = xim.reshape(B, 2, K * C_IN, T * S1W)
    consts = _prep_consts(
        np.asarray(W1, np.float32), np.asarray(b1, np.float32),
        np.asarray(W2, np.float32), np.asarray(b2, np.float32),
        np.asarray(Wfc, np.float32), np.asarray(bfc, np.float32),
    )
    if "nc" not in _CACHE:
        _CACHE["nc"] = _build()
    nc = _CACHE["nc"]

    in_maps = []
    for c in range(N_CORES):
        m = dict(consts)
        m["x"] = np.ascontiguousarray(xim[c * B_SH : (c + 1) * B_SH])
        in_maps.append(m)

    res = run_bass_kernel_spmd(
        nc, in_maps, core_ids=list(range(N_CORES)), trace=_trace
    )
    out = np.concatenate([res.results[c]["out"] for c in range(N_CORES)], axis=0)
    out = out.astype(np.float32)
    if _trace:
        return out, res
    return out


# revision 16
# speedup vs baseline: 1.0003x; 1.0003x over previous
"""Trainium2 Bass kernel for the ExportableStudentSNN1d problem.

v2: z-domain LIF (no per-step bias add), sigmoid-as-step spike masks with
free accum counting, {0,1} s1 encoding from the Pool engine.

Data-parallel over batch: 64 samples -> 8 cores x 8 samples. Each core runs
an identical NEFF on its batch shard; host concatenates the [8, 4] outputs.

Math notes (TAU1 = 1.0 makes layer-1 LIF memoryless):
  s1_t = (conv1(x_t)*G + b1*G >= TH1)  <=>  conv1(x_t) >= TH1/G - b1
  s1 stored {0,1} fp8 via Pool-engine is_ge (exact).
  Layer 2 runs in a 2^4-scaled, bias-shifted domain. With y = SC2*v2 and
  beta = A*b2 (A = (10/9)*G*SC2), define z = y - beta. Then
     z_t = conv2(s1_t; W2*A fp8) + alpha*chat_{t-1}        [all on PE]
     m_t = [z_t < theta - beta]   (theta = TH2*SC2)
         = Sigmoid(-BIG*(z_t - (theta-beta)))   on ACT; exact {0,1} by
           saturation for |arg|>=160 (measured); accum_out counts sum(m)
           per row -> spike counts for free.
     chat_t = (z_t + beta)*m_t    via scalar_tensor_tensor, fp8.
  alpha = fp8(-1/9) = -7/64 rides slot 1 of the tap-8 DoubleRow pair as
  alpha*I with chat as that slot's moving operand, so the LIF decay +
  carry recursion cost zero vector-engine work beyond the chat op.
  fp8 weight-quantization DC error is cancelled by folding
  -sum_{ci,k} dW[co,ci,k] * E[s1[ci]] into b2 (E[s1[ci]] analytic: conv1
  output is Gaussian with sigma = ||W1[ci]||_F).
  spikes = 1 - m, so out = (bfc + rowsum(Wfc)) - (Wfc @ sum(m))/(T*L).

conv2 is 5 fp8 DoubleRow matmuls per 512-chunk: taps (0,1),(2,3),(4,5),
(6,7) on slabs {0,2} + (tap8, alpha*I) on slabs {0,1+2h}. s1 tile is
[128, 4, S1P]: [s1, chat(h0), s1<<1, chat(h1)]; slab2 = slab0 shifted one
column via SBUF->SBUF DMA on the (otherwise idle) qSP queue; x staging
rides qAct so the per-step shift never queues behind a 4.5MB transfer.
conv1 is pipelined TWO steps ahead so the shift DMA latency (~2-4us) is
hidden at the ~2.6us steady-state cycle time. LIF ops are split per
512-chunk so the chat -> conv2(t+1) recursion never blocks the PE; the
h1/c1 chat chunk runs on Pool to keep DVE under the PE's cycle budget.
conv1 stays bf16 with a DMA-materialized im2col.
"""

import math

import numpy as np
import ml_dtypes

import concourse.bacc as bacc
import concourse.tile as tile
import concourse.mybir as mybir
from concourse.bass_utils import run_bass_kernel_spmd

F32 = mybir.dt.float32
BF16 = mybir.dt.bfloat16
FP8 = mybir.dt.float8e4
E4 = ml_dtypes.float8_e4m3

N_CORES = 8
B, C_IN, L, T = 64, 12, 2048, 20
C1, C2, K, PAD = 128, 256, 9, 4
GAIN, TAU2, TH1, TH2 = 3.0, 0.9, 0.02, 0.02
NCLS = 4
B_SH = B // N_CORES            # 8 samples per core
LH = 1024                      # L processed in halves
HALO = 8                       # x halo per side (conv1 then conv2 shifts)
S1W = LH + 2 * PAD             # 1032 s1 columns needed per L-half
S1P = 1040                     # s1 slab width (mult of 16)
A2S = (10.0 / 9.0) * GAIN      # 10/3: multiplier on conv2 psum
SC2 = 16.0                     # layer-2 scale; keeps z, chat in fp8 range
ALPHA = -7.0 / 64.0            # fp8(-1/9): carry decay inside the PE
BIG = 65536.0                  # sigmoid step sharpness (layer 2)
BIG1 = float(2 ** 26)          # sigmoid step sharpness (layer 1; narrower
                               # fractional band since s1 feeds 1152 taps)

_CACHE = {}


def _build():
    nc = bacc.Bacc("TRN2", target_bir_lowering=False, debug=False)

    # x arrives HOST-side im2col'd: row (12k+ci) of [b, lh] holds
    # x[b, ci, t, lh*1024 + c + k - 8] (zero-padded at L edges), so staging
    # a segment is ONE contiguous 4.5MB DMA instead of thousands of 2KB
    # descriptors that saturate the rings.
    x_d = nc.dram_tensor(
        "x", [B_SH, 2, K * C_IN, T * S1W], BF16, kind="ExternalInput")
    w1t_d = nc.dram_tensor("w1t", [K * C_IN, C1], BF16, kind="ExternalInput")
    # 10 SwInterleave stationaries: groups j*2+h (tap pairs) and 8+h
    # (tap8 + alpha*I); per partition flat = [A127,B127,A126,B126,...,B0]
    w2sw_d = nc.dram_tensor("w2sw", [C1, 10, 2 * C1], FP8, kind="ExternalInput")
    nth1s_d = nc.dram_tensor("nth1s", [C1, 1], F32, kind="ExternalInput")
    beta_d = nc.dram_tensor("beta", [C1, 2], F32, kind="ExternalInput")
    sigb_d = nc.dram_tensor("sigb", [C1, 2], F32, kind="ExternalInput")
    wfc_d = nc.dram_tensor("wfc", [C1, 2 * NCLS], F32, kind="ExternalInput")
    bfc_d = nc.dram_tensor("bfc", [NCLS, 1], F32, kind="ExternalInput")
    out_d = nc.dram_tensor("out", [B_SH, NCLS], F32, kind="ExternalOutput")

    with tile.TileContext(nc) as tc:
        with (
            tc.tile_pool(name="const", bufs=1) as cpool,
            tc.tile_pool(name="xstage", bufs=2) as xpool,
            tc.tile_pool(name="s1", bufs=4) as s1pool,
            tc.tile_pool(name="lif", bufs=4) as lifpool,
            tc.tile_pool(name="psum1", bufs=1, space="PSUM") as pp1,
            tc.tile_pool(name="psum2", bufs=2, space="PSUM") as pp2,
            tc.tile_pool(name="psfc", bufs=1, space="PSUM") as ppfc,
        ):
            # ---- constants / weights (resident) ----
            # w1t rows (12k+ci) hold W1[:, ci, k] (im2col layout)
            w1t = cpool.tile([K * C_IN, C1], BF16)
            nc.sync.dma_start(w1t[:], w1t_d.ap())
            # SwInterleave DR stationaries: contiguous A/B-interleaved,
            # column-reversed layout so LDWEIGHTS streams contiguously
            w2sw = cpool.tile([C1, 10, 2 * C1], FP8)
            nc.sync.dma_start(w2sw[:], w2sw_d.ap())
            nth1s = cpool.tile([C1, 1], F32)
            nc.sync.dma_start(nth1s[:], nth1s_d.ap())
            beta = cpool.tile([C1, 2], F32)
            nc.sync.dma_start(beta[:], beta_d.ap())
            sigb = cpool.tile([C1, 2], F32)
            nc.sync.dma_start(sigb[:], sigb_d.ap())
            wfc = cpool.tile([C1, 2 * NCLS], F32)
            nc.sync.dma_start(wfc[:], wfc_d.ap())
            bfc = cpool.tile([NCLS, 1], F32)
            nc.sync.dma_start(bfc[:], bfc_d.ap())
            # m01 sums, one column per (h, b, lh, t)
            NACC = 2 * B_SH * 2 * T
            acc = cpool.tile([C1, NACC], F32)

            segs = [(b, lh) for b in range(B_SH) for lh in range(2)]
            n_items = len(segs) * T

            def stage_segment(idx):
                # host-side im2col: one contiguous DMA per segment, on the
                # qAct queue so the per-step s1 shift (qSP) never queues
                # behind it
                b, lh = segs[idx]
                xs = xpool.tile([K * C_IN, T * S1W], BF16)
                src = x_d.ap()[b, lh]
                # split every segment: the first conv1 needs only t=0/1,
                # so it must not wait on the full 4.5MB transfer's semaphore
                nc.scalar.dma_start(xs[:, 0 : 2 * S1W], src[:, 0 : 2 * S1W])
                nc.scalar.dma_start(xs[:, 2 * S1W :], src[:, 2 * S1W :])
                return xs

            def conv1_mm(xs, t):
                # conv1: K=108 bf16 matmuls per chunk, emitted EARLY in the
                # cycle so the PE does them before conv2
                p1 = pp1.tile([C1, 1536], F32)
                for c0, cn in ((0, 512), (512, 512), (1024, S1W - 1024)):
                    nc.tensor.matmul(
                        p1[:, c0 : c0 + cn],
                        w1t[:],
                        xs[:, t * S1W + c0 : t * S1W + c0 + cn],
                        start=True,
                        stop=True,
                    )
                return p1

            def s1_make(p1, init_zero=False):
                # s1 slab0 = {0,1} fp8 sigmoid step (exact by saturation).
                # Emitted AFTER the cycle's m01 ops so it sits last in the
                # ACT queue (it has 2 cycles of slack; the m01s have none).
                # slab2 = slab0 shifted one column via qSP SBUF->SBUF DMA
                # (2 cycles of runway from the 2-ahead pipeline).
                # Cols >= 1032 are junk but never read by conv2.
                s1 = s1pool.tile([C1, 4, S1P], FP8)
                if init_zero:
                    # only the first prologue tile needs memsets; later
                    # segment-start tiles get their carry slabs zeroed by
                    # the t=19 z*0 writes
                    nc.gpsimd.memset(s1[:, 1], 0.0)
                    nc.gpsimd.memset(s1[:, 3], 0.0)
                nc.scalar.activation(
                    s1[:, 0], p1[:, 0:S1P],
                    mybir.ActivationFunctionType.Sigmoid,
                    bias=nth1s[:], scale=BIG1,
                )
                nc.sync.dma_start(s1[:, 2, 0 : S1P - 1], s1[:, 0, 1:S1P])
                return s1

            def conv2_block(s1, h):
                # 5 DR matmuls per 512-chunk: 4 tap pairs on slabs {0,2} +
                # the (tap8, alpha*I) pair on slabs {0, 1+2h} -- slot 1's
                # moving operand is the previous step's carry chat(h).
                p2 = pp2.tile([C1, LH], F32)
                for c0 in (0, 512):
                    for j in range(4):
                        nc.tensor.matmul(
                            p2[:, c0 : c0 + 512],
                            w2sw[:, j * 2 + h, :],
                            s1[:, 0:3:2, c0 + 2 * j : c0 + 2 * j + 512],
                            start=(j == 0),
                            stop=False,
                            perf_mode=mybir.MatmulPerfMode.DoubleRowSwInterleave,
                        )
                    nc.tensor.matmul(
                        p2[:, c0 : c0 + 512],
                        w2sw[:, 8 + h, :],
                        (s1[:, 0:2:1, c0 + 8 : c0 + 8 + 512] if h == 0
                         else s1[:, 0:4:3, c0 + 8 : c0 + 8 + 512]),
                        start=False,
                        stop=True,
                        perf_mode=mybir.MatmulPerfMode.DoubleRowSwInterleave,
                    )
                return p2

            def lif_step(p2, s1_next, h, col, zero=False):
                # m = sigmoid-step on ACT, full width (accum -> counts);
                # chat = (z + beta)*m on DVE per 512-chunk (so the
                # chat -> conv2(t+1) recursion never blocks the PE),
                # written into the NEXT s1 tile's carry slab at +8
                # (tap-8 window alignment).
                m01 = lifpool.tile([C1, LH], F32)
                nc.scalar.activation(
                    m01[:], p2[:],
                    mybir.ActivationFunctionType.Sigmoid,
                    bias=sigb[:, h : h + 1],
                    scale=-BIG,
                    accum_out=acc[:, col + h * (NACC // 2) :
                                  col + h * (NACC // 2) + 1],
                )
                if s1_next is None:
                    return
                for c0 in (0, 512):
                    dst = s1_next[:, 1 + 2 * h, 8 + c0 : 8 + c0 + 512]
                    if zero:
                        # segment start: v2 resets, write exact zeros
                        nc.vector.tensor_scalar(
                            dst, p2[:, c0 : c0 + 512], 0.0, None,
                            op0=mybir.AluOpType.mult,
                        )
                    else:
                        nc.vector.scalar_tensor_tensor(
                            dst, p2[:, c0 : c0 + 512],
                            beta[:, h : h + 1], m01[:, c0 : c0 + 512],
                            op0=mybir.AluOpType.add,
                            op1=mybir.AluOpType.mult,
                        )

            # conv1 runs TWO items ahead of conv2/LIF so the s1 sigmoid and
            # the qSP shift DMA are always done before the PE needs slab0/2
            staged = {0: stage_segment(0)}
            tiles = []

            def emit_conv1_mm(j):
                jdx, jt = divmod(j, T)
                return conv1_mm(staged[jdx], jt)

            tiles.append(s1_make(emit_conv1_mm(0), init_zero=True))
            tiles.append(s1_make(emit_conv1_mm(1)))
            for idx in range(len(segs)):
                b, lh = segs[idx]
                if idx + 1 < len(segs):
                    staged[idx + 1] = stage_segment(idx + 1)
                staged.pop(idx - 1, None)
                for t in range(T):
                    i = idx * T + t
                    p1_next = emit_conv1_mm(i + 2) if i + 2 < n_items else None
                    s1_cur = tiles[i]
                    s1_next = tiles[i + 1] if i + 1 < n_items else None
                    tiles[i] = None
                    col = b * (2 * T) + lh * T + t
                    last_t = t == T - 1
                    p2_0 = conv2_block(s1_cur, 0)
                    lif_step(p2_0, s1_next, 0, col, zero=last_t)
                    p2_1 = conv2_block(s1_cur, 1)
                    lif_step(p2_1, s1_next, 1, col, zero=last_t)
                    if p1_next is not None:
                        tiles.append(s1_make(p1_next))

            # ---- pooling + FC head ----
            # pooled = sum over (lh, t, chunk) of m01 row-sums per (h, b)
            pooled = cpool.tile([C1, 2 * B_SH], F32)
            nc.vector.tensor_reduce(
                pooled[:],
                acc[:].rearrange("p (h b c) -> p (h b) c", h=2, b=B_SH),
                axis=mybir.AxisListType.X, op=mybir.AluOpType.add,
            )
            pfc = ppfc.tile([NCLS, B_SH], F32)
            for h in range(2):
                nc.tensor.matmul(
                    pfc[:],
                    wfc[:, h * NCLS : (h + 1) * NCLS],
                    pooled[:, h * B_SH : (h + 1) * B_SH],
                    start=(h == 0),
                    stop=(h == 1),
                )
            # pfc holds Wfc @ sum(m); spikes = 1 - m is folded into scale
            # and the host-adjusted bias: out = bfcc - pfc/(T*L)
            fin = cpool.tile([NCLS, B_SH], F32)
            nc.scalar.activation(
                fin[:], pfc[:], mybir.ActivationFunctionType.Identity,
                bias=bfc[:], scale=-1.0 / float(T * L),
            )
            nc.sync.dma_start(out_d.ap().rearrange("b c -> c b"), fin[:])

    nc.compile()
    return nc


def _prep_consts(W1, b1, W2, b2, Wfc, bfc):
    # w1t im2col layout: row (12k+ci), col co = W1[co, ci, k]
    w1t = np.ascontiguousarray(W1.transpose(2, 1, 0)).reshape(K * C_IN, C1)
    # W2 pre-scaled to the 2^4 domain, fp8-e4m3 quantized
    A = A2S * SC2
    w2q8 = (W2.astype(np.float64) * A).astype(np.float32).astype(E4)
    wt = np.ascontiguousarray(w2q8.transpose(1, 2, 0))  # [C1, K, C2]
    # SwInterleave layout per group g, per partition:
    # flat[2k] = A[:, 127-k], flat[2k+1] = B[:, 127-k]
    # (A = slot0 weights, B = slot1 weights, columns reversed)
    eye = (np.eye(C1, dtype=np.float32) * ALPHA).astype(E4).astype(np.float32)
    w2sw = np.zeros((C1, 10, 2 * C1), dtype=E4)
    for h in range(2):
        hs = slice(h * C1, (h + 1) * C1)
        for j in range(4):
            wa = wt[:, 2 * j, hs].astype(np.float32)
            wb = wt[:, 2 * j + 1, hs].astype(np.float32)
            w2sw[:, j * 2 + h, 0::2] = wa[:, ::-1].astype(E4)
            w2sw[:, j * 2 + h, 1::2] = wb[:, ::-1].astype(E4)
        wa = wt[:, 8, hs].astype(np.float32)
        w2sw[:, 8 + h, 0::2] = wa[:, ::-1].astype(E4)
        w2sw[:, 8 + h, 1::2] = eye[:, ::-1].astype(E4)
    # DC correction: E[s1[ci]] = Phi((b1 - TH1/G)/sigma), sigma = ||W1[ci]||
    sig = np.sqrt((W1.astype(np.float64) ** 2).sum(axis=(1, 2)))
    z = (b1.astype(np.float64) - TH1 / GAIN) / sig
    p_ci = np.array([0.5 * (1.0 + math.erf(v / math.sqrt(2.0))) for v in z])
    dw = w2q8.astype(np.float64) / A - W2.astype(np.float64)
    b2c = b2.astype(np.float64) - np.einsum("oik,i->o", dw, p_ci)
    # layer-1 threshold: s1 = [conv1 >= TH1/G - b1] via sigmoid step
    nth1s = (-BIG1 * (TH1 / GAIN - b1)).reshape(C1, 1).astype(np.float32)
    beta_full = (A * b2c).astype(np.float32)              # [C2]
    theta = TH2 * SC2
    sigb_full = (BIG * (theta - beta_full)).astype(np.float32)
    beta = beta_full.reshape(2, C1).T.copy()              # [128, 2]
    sigb = sigb_full.reshape(2, C1).T.copy()              # [128, 2]
    wfcT = Wfc.T.reshape(2, C1, NCLS)                     # [2, 128, 4]
    wfc_t = wfcT.transpose(1, 0, 2).reshape(C1, 2 * NCLS).copy()
    # spikes = 1 - m folded into the FC epilogue:
    # out = (bfc + rowsum(Wfc)) - (Wfc @ sum_m)/(T*L)
    bfc_c = (bfc + Wfc.sum(axis=1)).reshape(NCLS, 1).astype(np.float32)
    return {
        "w1t": w1t.astype(ml_dtypes.bfloat16),
        "w2sw": w2sw,
        "nth1s": nth1s,
        "beta": beta,
        "sigb": sigb,
        "wfc": wfc_t.astype(np.float32),
        "bfc": bfc_c,
    }


def kernel(x, W1, b1, W2, b2, Wfc, bfc, _trace=False):
    x = np.asarray(x, dtype=np.float32)
    # [B, Cin, L, T] -> [B, Cin, T, L] bf16 so on-chip reads are unit-stride
    x_t = np.ascontiguousarray(x.transpose(0, 1, 3, 2)).astype(ml_dtypes.bfloat16)
    # host-side im2col: xim[b, lh, 12k+ci, t, c] = x[b, ci, t, lh*LH+c+k-8]
    # (zero outside [0, L)) -- turns on-chip staging into one big DMA
    xim = np.zeros((B, 2, K * C_IN, T, S1W), dtype=ml_dtypes.bfloat16)
    for lh in range(2):
        l0 = lh * LH
        for k in range(K):
            c_lo = max(0, HALO - k - l0)
            c_hi = min(S1W, L - l0 - k + HALO)
            xim[:, lh, C_IN * k : C_IN * (k + 1), :, c_lo:c_hi] = (
                x_t[:, :, :, l0 + c_lo + k - HALO : l0 + c_hi + k - HALO])
    xim = xim.reshape(B, 2, K * C_IN, T * S1W)
    consts = _prep_consts(
        np.asarray(W1, np.float32), np.asarray(b1, np.float32),
        np.asarray(W2, np.float32), np.asarray(b2, np.float32),
        np.asarray(Wfc, np.float32), np.asarray(bfc, np.float32),
    )
    if "nc" not in _CACHE:
        _CACHE["nc"] = _build()
    nc = _CACHE["nc"]

    in_maps = []
    for c in range(N_CORES):
        m = dict(consts)
        m["x"] = np.ascontiguousarray(xim[c * B_SH : (c + 1) * B_SH])
        in_maps.append(m)

    res = run_bass_kernel_spmd(
        nc, in_maps, core_ids=list(range(N_CORES)), trace=_trace
    )
    out = np.concatenate([res.results[c]["out"] for c in range(N_CORES)], axis=0)
    out = out.astype(np.float32)
    if _trace:
        return out, res
    return out
